# revision 39
# baseline (speedup 1.0000x reference)
"""DeepSeekV3 MLA attention kernel for Trainium2 (8 NeuronCores, Bass/Tile).

Sharding: core c -> batch b = c // 4, head-group g = c % 4 (8 of 32 heads).
Each core runs the layer for its batch restricted to its heads and emits a
partial o_proj output [2048, 4096] (bf16); the host sums the 4 partials per
batch in f32.

The shared a-projections (q_a, kv_a) are *sequence-parallel*: each core
computes and RMS-normalizes the latents for its 512-token slice only, then
AllGathers over the 4-core batch group replicate the full normalized latents.
The kv gather fires first (small, unblocks kv_b early); the q gather is split
in two so wire time pipelines with the tail of the q_a compute.  A tiny dummy
AllGather is issued before any compute to absorb the one-time collective
entry barrier (~47us) underneath phase-A compute.

Layouts (feature-major, [128, chunks, tokens]); all matmul operands bf16
(full PE rate), PSUM accumulation fp32:
  - x is fed transposed (xT [4096, 2048]); matmuls contract over the
    partition dim with N = 512 token tiles (one PSUM bank).
  - RoPE halves are packed [4*lo(32) | 4*hi(32)] per 4 heads; the rotated
    output is stored to DRAM in its natural [lo/hi, ci, 128, tb] layout and
    attention re-gathers the 4 32-row groups per head pair with four block
    DMAs (replaces the former 16-way scatter-DMA storm per tile).
    Head pairs (2j, 2j+1) land in partition halves [0:64], [64:128] of the
    attention rope tile, so attention row-packs the two K=64 rope matmuls
    into one PE pass via tile_position (concurrent sub-array execution).
  - Softmax skips the max-subtraction (scores are O(5), exp safe in fp32).
    Denominators: exp tiles are summed with a bf16 pairwise tree on the DVE
    (binary counter of partials, no serial chain, 2x 16-bit mode) and
    reduced with a single ones-matmul per (head, query-tile) - removes the
    former per-key-tile ones matmuls (~8.6 GFLOP of PE time).  1/den uses
    reciprocal_approx_fast (~5x cheaper, accuracy far below bf16 noise).
    The score->exp->pv chain is software-pipelined two key-tiles deep so
    the PE never waits on the ACT exp (keeps the HAM clock-gate warm).
  - Phase weights (kv_b, q_b, o_w) are prefetched one phase ahead, phase
    input streams (A2's x tiles, C/B latents, attention j=0 tiles) are
    enqueued before the previous phase's data-dependent stores so no phase
    opens behind a head-of-line-blocked DMA queue.
"""

import math

import numpy as np

try:
    import concourse.bacc as bacc  # noqa: F401
except ImportError:
    import sys

    for _p in ("/root/.axon_site/_ro/trn_rl_repo", "/opt/trn_rl_repo"):
        if _p not in sys.path:
            sys.path.insert(0, _p)

import concourse.bacc as bacc
import concourse.mybir as mybir
import concourse.tile as tile
from concourse.bass_utils import run_bass_kernel_spmd

# model dims
H, DN, DR, DV = 32, 128, 64, 128
HID, QR, KVR = 4096, 1536, 512
EPS, MAXP = 1e-6, 4096
B, S = 2, 2048
P = 128
TT = 512  # token tile (matmul moving dim)
NH = 8  # heads per core
NCORES = 8
SCALE = 1.0 / math.sqrt(DN + DR)
HIDC = HID // P  # 32
QRC = QR // P  # 12
KVRC = KVR // P  # 4

F32 = mybir.dt.float32
BF16 = mybir.dt.bfloat16
NP_BF16 = mybir.dt.np(BF16)

EXP_FN = mybir.ActivationFunctionType.Exp
SQRT_FN = mybir.ActivationFunctionType.Sqrt

SEQPAR = True  # sequence-parallel a-projections via AllGather
QSPLIT = 6  # q latent chunks in the first of the two q AllGathers


def build_nc(tb=S, seqpar=SEQPAR):
    """Build the per-core Bass program (same program on all 8 cores)."""
    import os as _os
    phmax = int(_os.environ.get("PHMAX", "9"))
    ntt = tb // TT  # token tiles for phases B..E
    ntc = tb // P  # token chunks
    tta = tb // 4  # a-projection slice length per core
    nc = bacc.Bacc("TRN2", target_bir_lowering=False, debug=False,
                   num_devices=NCORES)

    xT = nc.dram_tensor("xT", [HID, tb], BF16, kind="ExternalInput")
    qa_wT = nc.dram_tensor("qa_wT", [HID, QR], BF16, kind="ExternalInput")
    kva_wT = nc.dram_tensor("kva_wT", [HID, KVR], BF16, kind="ExternalInput")
    kr_wT = nc.dram_tensor("kr_wT", [HID, NH * DR], BF16, kind="ExternalInput")
    qb_wT = nc.dram_tensor("qb_wT", [QR, NH * (DN + DR)], BF16,
                           kind="ExternalInput")
    kvbk_wT = nc.dram_tensor("kvbk_wT", [KVR, NH * DN], BF16,
                             kind="ExternalInput")
    kvbv_wT = nc.dram_tensor("kvbv_wT", [KVR, NH * DV], BF16,
                             kind="ExternalInput")
    o_wT = nc.dram_tensor("o_wT", [NH * DV, HID], BF16, kind="ExternalInput")
    cos_in = nc.dram_tensor("cos_rep", [P, tb], BF16, kind="ExternalInput")
    sin_in = nc.dram_tensor("sin_rep", [P, tb], BF16, kind="ExternalInput")
    if seqpar:
        xA = nc.dram_tensor("xA", [HID, tta], BF16, kind="ExternalInput")
    out_part = nc.dram_tensor("out_part", [tb, HID], BF16,
                              kind="ExternalOutput")

    x_ap = xT[:, :].rearrange("(c p) t -> p c t", p=P)
    qa_ap = qa_wT[:, :].rearrange("(c p) m -> p c m", p=P)
    kva_ap = kva_wT[:, :].rearrange("(c p) m -> p c m", p=P)
    kr_ap = kr_wT[:, :].rearrange("(c p) m -> p c m", p=P)
    qb_ap = qb_wT[:, :].rearrange("(c p) m -> p c m", p=P)
    kvbk_ap = kvbk_wT[:, :].rearrange("(c p) m -> p c m", p=P)
    kvbv_ap = kvbv_wT[:, :].rearrange("(c p) m -> p c m", p=P)
    ow_ap = o_wT[:, :].rearrange("(c p) m -> p c m", p=P)
    if seqpar:
        xa_ap = xA[:, :].rearrange("(c p) t -> p c t", p=P)

    with tile.TileContext(nc) as tc:
        with tc.tile_pool(name="const", bufs=1) as constp, \
             tc.tile_pool(name="dram", bufs=1, space="DRAM") as dram:
            ones_f = constp.tile([P, P], F32)
            nc.any.memset(ones_f[:], 1.0)
            ones_b = constp.tile([P, P], BF16)
            nc.vector.tensor_copy(out=ones_b[:], in_=ones_f[:])
            eps_sb = constp.tile([P, 1], F32)
            nc.any.memset(eps_sb[:], EPS)

            # dummy collective: absorbs the one-time CC entry barrier under
            # phase-A compute (the real gathers then start immediately).
            if seqpar:
                dummy_in = dram.tile([P, 1], BF16)
                dummy_out = dram.tile([4 * P, 1], BF16)
                nc.gpsimd.collective_compute(
                    "AllGather", mybir.AluOpType.bypass,
                    replica_groups=[[0, 1, 2, 3], [4, 5, 6, 7]],
                    ins=[dummy_in.opt()], outs=[dummy_out.opt()])

            # gathered normalized latents: block g = tokens [g*tta,(g+1)*tta)
            latq_in = dram.tile([QRC, P, tta], BF16)
            latq_all_a = dram.tile([4 * QSPLIT, P, tta], BF16)
            latq_all_b = dram.tile([4 * (QRC - QSPLIT), P, tta], BF16)
            latkv_in = dram.tile([KVRC, P, tta], BF16)
            latkv_all = dram.tile([4 * KVRC, P, tta], BF16)
            qnope_d = dram.tile([P, NH, tb], BF16)
            knope_d = dram.tile([P, NH, tb], BF16)
            # rope in natural producer layout: [lo/hi, ci, 128, tb]
            qrope_d = dram.tile([2, 2, P, tb], BF16)
            krope_d = dram.tile([2, 2, P, tb], BF16)
            v_d = dram.tile([P, ntc, NH * DV], BF16)
            attn_d = dram.tile([P, NH, tb], BF16)

            def rope_evict(lo_src, hi_src, tsl, pool, cos_sb, sin_sb, tag):
                """lo/hi chunk pair [P, n] (4 heads x 32 rows) -> rotate."""
                t1 = pool.tile([P, TT], F32, tag=tag, name="rt1")
                t2 = pool.tile([P, TT], F32, tag=tag, name="rt2")
                n = tsl.stop - tsl.start
                nc.vector.tensor_mul(out=t1[:, :n], in0=lo_src[:],
                                     in1=cos_sb[:, tsl])
                nc.vector.tensor_mul(out=t2[:, :n], in0=hi_src[:],
                                     in1=sin_sb[:, tsl])
                lo_o = pool.tile([P, TT], BF16, tag=tag, name="rlo")
                nc.vector.tensor_sub(out=lo_o[:, :n], in0=t1[:, :n],
                                     in1=t2[:, :n])
                t3 = pool.tile([P, TT], F32, tag=tag, name="rt3")
                t4 = pool.tile([P, TT], F32, tag=tag, name="rt4")
                nc.vector.tensor_mul(out=t3[:, :n], in0=hi_src[:],
                                     in1=cos_sb[:, tsl])
                nc.vector.tensor_mul(out=t4[:, :n], in0=lo_src[:],
                                     in1=sin_sb[:, tsl])
                hi_o = pool.tile([P, TT], BF16, tag=tag, name="rhi")
                nc.vector.tensor_add(out=hi_o[:, :n], in0=t3[:, :n],
                                     in1=t4[:, :n])
                return lo_o, hi_o

            def rope_store(lo_o, hi_o, ci, dst_d, tsl):
                n = tsl.stop - tsl.start
                nc.sync.dma_start(out=dst_d[0, ci, :, tsl], in_=lo_o[:, :n])
                nc.sync.dma_start(out=dst_d[1, ci, :, tsl], in_=hi_o[:, :n])

            def rope_load(dst, src_d, j):
                """Gather pair j's rope rows [e-lo, e-hi, o-lo, o-hi]."""
                ci, hh0 = j // 2, (j % 2) * 2
                for r, (lh, hh) in enumerate(
                        ((0, hh0), (1, hh0), (0, hh0 + 1), (1, hh0 + 1))):
                    nc.sync.dma_start(
                        out=dst[32 * r:32 * r + 32, :],
                        in_=src_d[lh, ci, 32 * hh:32 * hh + 32, :])

            # pools for the A2 input stream, allocated under the A1 pools so
            # their DMAs can be emitted inside A1 (ahead of the q-latent
            # stores, which only execute once A1's PE stream finishes --
            # emitting A2's loads after them would head-of-line block A2)
            ropep = tc.alloc_tile_pool(name="ropetab", bufs=1)
            xpool = tc.alloc_tile_pool(name="krx", bufs=33)
            krwp = tc.alloc_tile_pool(name="krw", bufs=4)
            xt_pre, krw_tiles, rope_tabs = [], [], []

            def emit_a2_prefetch():
                for k in range(HIDC):
                    xt = xpool.tile([P, TT], BF16, tag="krx",
                                    name=f"krx0_{k}")
                    nc.sync.dma_start(out=xt[:], in_=x_ap[:, k, 0:TT])
                    xt_pre.append(xt)
                cos_sb = ropep.tile([P, tb], BF16)
                sin_sb = ropep.tile([P, tb], BF16)
                nc.sync.dma_start(out=cos_sb[:], in_=cos_in[:, :])
                nc.sync.dma_start(out=sin_sb[:], in_=sin_in[:, :])
                rope_tabs.extend([cos_sb, sin_sb])
                for m in range(4):
                    wt = krwp.tile([P, HIDC, P], BF16, tag="krw",
                                   name=f"krw{m}")
                    for pt in range(4):
                        ks = slice(8 * pt, 8 * (pt + 1))
                        nc.sync.dma_start(out=wt[:, ks, :],
                                          in_=kr_ap[:, ks, m * P:(m + 1) * P])
                    krw_tiles.append(wt)

            # ------- Phase A1: seq-sliced q_a/kv_a + rms-norm + gather -------
            # groups of output chunks; contraction over HID (32 k-chunks)
            with tc.tile_pool(name="apw", bufs=7) as wpool, \
                 tc.tile_pool(name="apx", bufs=1 if seqpar else 2) as axp, \
                 tc.tile_pool(name="apraw", bufs=QRC + KVRC + 1) as rawp, \
                 tc.tile_pool(name="apev", bufs=8) as evp, \
                 tc.tile_pool(name="apacc", bufs=6, space="PSUM") as accp, \
                 tc.tile_pool(name="apstat", bufs=2, space="PSUM") as statp:
                slices = range(1) if seqpar else range(4)
                if phmax < 1:
                    slices = range(0)

                def norm_and_ship(kind, raws, stats, g4, m0, m1):
                    rank = QR if kind == "q" else KVR
                    sdev = evp.tile([P, tta], F32, tag="ev")
                    nc.scalar.activation(sdev[:], stats[kind][:], SQRT_FN,
                                         bias=eps_sb[:], scale=1.0 / rank)
                    rstd = evp.tile([P, tta], F32, tag="ev")
                    nc.vector.reciprocal(rstd[:], sdev[:])
                    for m in range(m0, m1):
                        nrm = evp.tile([P, tta], BF16, tag="ev")
                        nc.vector.tensor_mul(out=nrm[:],
                                             in0=raws[(kind, m)][:],
                                             in1=rstd[:])
                        if seqpar:
                            l_in = latq_in if kind == "q" else latkv_in
                            nc.sync.dma_start(out=l_in[m], in_=nrm[:])
                        else:
                            if kind == "kv":
                                nc.sync.dma_start(
                                    out=latkv_all[g4 * KVRC + m], in_=nrm[:])
                            elif m < QSPLIT:
                                nc.sync.dma_start(
                                    out=latq_all_a[g4 * QSPLIT + m],
                                    in_=nrm[:])
                            else:
                                nc.sync.dma_start(
                                    out=latq_all_b[
                                        g4 * (QRC - QSPLIT) + m - QSPLIT],
                                    in_=nrm[:])

                def gather(ins, outs):
                    nc.gpsimd.collective_compute(
                        "AllGather", mybir.AluOpType.bypass,
                        replica_groups=[[0, 1, 2, 3], [4, 5, 6, 7]],
                        ins=[ins.opt()], outs=[outs.opt()])

                for g4 in slices:
                    asl = slice(g4 * tta, (g4 + 1) * tta)
                    xs = axp.tile([P, HIDC, tta], BF16, tag="apx",
                                    name=f"xs{g4}")

                    def xs_load(k):
                        if seqpar:
                            nc.sync.dma_start(out=xs[:, k, :],
                                              in_=xa_ap[:, k, :])
                        else:
                            nc.sync.dma_start(out=xs[:, k, :],
                                              in_=x_ap[:, k, asl])

                    # stage the first k-chunks + first weight parts in small
                    # DMAs so the opening matmul is behind <1MB of queue
                    for k in range(2):
                        xs_load(k)
                    first_group = True
                    raws = {}
                    stats = {}
                    # rms-norm stat accumulated per kind across all m-groups
                    for kind, m0, m1 in [("kv", 0, KVRC), ("q", 0, QSPLIT),
                                         ("q", QSPLIT, QRC)]:
                        src = qa_ap if kind == "q" else kva_ap
                        nm = QRC if kind == "q" else KVRC
                        wts = []
                        for m in range(m0, m1):
                            wt = wpool.tile([P, HIDC, P], BF16, tag="apw",
                                            name=f"apw{kind}{g4}_{m}")
                            if not first_group:
                                nc.sync.dma_start(
                                    out=wt[:],
                                    in_=src[:, :, m * P:(m + 1) * P])
                            wts.append(wt)
                        if first_group:
                            # part-major order: k 0-7 of every m arrives
                            # before any deeper k chunk
                            for pt in range(4):
                                ks = slice(8 * pt, 8 * (pt + 1))
                                for mi, m in enumerate(range(m0, m1)):
                                    nc.sync.dma_start(
                                        out=wts[mi][:, ks, :],
                                        in_=src[:, ks, m * P:(m + 1) * P])
                                if pt == 0:
                                    for k in range(2, 8):
                                        xs_load(k)
                            for k in range(8, HIDC):
                                xs_load(k)
                            first_group = False
                        accs = [accp.tile([P, tta], F32, tag="acc",
                                          name=f"acc{kind}{g4}_{m}")
                                for m in range(m0, m1)]
                        for k in range(HIDC):
                            for mi in range(m1 - m0):
                                nc.tensor.matmul(
                                    accs[mi][:], wts[mi][:, k, :], xs[:, k, :],
                                    start=(k == 0), stop=(k == HIDC - 1))
                        if kind not in stats:
                            stats[kind] = statp.tile(
                                [P, tta], F32, tag="stat",
                                name=f"stat{kind}_{g4}")
                        stat = stats[kind]
                        for mi, m in enumerate(range(m0, m1)):
                            raw = rawp.tile([P, tta], BF16, tag="raw",
                                            name=f"raw{kind}{g4}_{m}")
                            nc.vector.tensor_copy(out=raw[:], in_=accs[mi][:])
                            raws[(kind, m)] = raw
                            sq = evp.tile([P, tta], BF16, tag="ev")
                            nc.vector.tensor_mul(out=sq[:], in0=raw[:],
                                                 in1=raw[:])
                            nc.tensor.matmul(stat[:], ones_b[:], sq[:],
                                             start=(m == 0), stop=(m == nm - 1))
                        if kind == "kv":
                            norm_and_ship("kv", raws, stats, g4, 0, KVRC)
                            if seqpar and phmax >= 1 and g4 == slices[-1]:
                                gather(latkv_in, latkv_all)
                        elif m1 == QRC:
                            # A2's input stream enqueues here: after all A1
                            # weight loads, before the q-latent stores
                            if not xt_pre:
                                emit_a2_prefetch()
                            # second q half: stat now complete -> norm all q
                            norm_and_ship("q", raws, stats, g4, 0, QSPLIT)
                            if seqpar and phmax >= 1 and g4 == slices[-1]:
                                gather(latq_in[0:QSPLIT], latq_all_a)
                            norm_and_ship("q", raws, stats, g4, QSPLIT, QRC)
                            if seqpar and phmax >= 1 and g4 == slices[-1]:
                                gather(latq_in[QSPLIT:QRC], latq_all_b)

            def load_lat(pool, tag, name, kind, nk, t):
                """Load latent chunks [0, nk) for token tile t."""
                tiles = []
                for kk in range(nk):
                    qn = pool.tile([P, TT], BF16, tag=tag,
                                   name=f"{name}{t}_{kk}")
                    if kind == "q":
                        nm_a, nm_b = QSPLIT, QRC - QSPLIT
                    for bk in range(4):
                        lo, hi = bk * tta, (bk + 1) * tta
                        if lo >= t * TT and hi <= (t + 1) * TT:
                            if kind == "kv":
                                src = latkv_all[bk * KVRC + kk]
                            elif kk < QSPLIT:
                                src = latq_all_a[bk * nm_a + kk]
                            else:
                                src = latq_all_b[bk * nm_b + kk - QSPLIT]
                            nc.sync.dma_start(
                                out=qn[:, lo - t * TT:hi - t * TT], in_=src)
                    tiles.append(qn)
                return tiles

            # ------- Phase A2: k_rope (all tb tokens, this core's heads) -----
            if not xt_pre:  # phmax debug path: A1 skipped entirely
                emit_a2_prefetch()
            cos_sb, sin_sb = rope_tabs
            wts = krw_tiles
            # second x pool in the space A1 just freed: gives the t>=1 x
            # stream real DMA lookahead (two full tiles + rotation; a
            # 36-slot pool still gated t>=2 loads on PE progress)
            xpool2 = tc.alloc_tile_pool(name="krx2", bufs=68)
            xt_pre2 = {}
            if phmax >= 2:
                for t in (1, 2):
                    if t >= ntt:
                        break
                    for k in range(HIDC):
                        xt = xpool2.tile([P, TT], BF16, tag="krx2",
                                         name=f"krx{t}_{k}")
                        nc.sync.dma_start(
                            out=xt[:], in_=x_ap[:, k, t * TT:(t + 1) * TT])
                        xt_pre2[(t, k)] = xt
            kvwp = tc.alloc_tile_pool(name="kvw", bufs=1, side="right")
            kvnp = tc.alloc_tile_pool(name="kvn", bufs=2 * KVRC + 1,
                                      side="right")

            with tc.tile_pool(name="krev", bufs=10) as evp, \
                 tc.tile_pool(name="kracc", bufs=5, space="PSUM") as accp:
                kvn_pre = {}
                for t in range(ntt if phmax >= 2 else 0):
                    tsl = slice(t * TT, (t + 1) * TT)
                    accs = [accp.tile([P, TT], F32, tag="acc",
                                      name=f"kracc{t}_{m}") for m in range(4)]
                    for k in range(HIDC):
                        if t == 0:
                            xt = xt_pre[k]
                        elif (t, k) in xt_pre2:
                            xt = xt_pre2[(t, k)]
                        else:
                            xt = xpool2.tile([P, TT], BF16, tag="krx2",
                                             name=f"krx{t}_{k}")
                            nc.sync.dma_start(out=xt[:], in_=x_ap[:, k, tsl])
                        for mi in range(4):
                            nc.tensor.matmul(
                                accs[mi][:], wts[mi][:, k, :], xt[:],
                                start=(k == 0), stop=(k == HIDC - 1))
                    # chunks [lo0, lo1, hi0, hi1] -> rope
                    for ci in range(2):
                        lo_o, hi_o = rope_evict(accs[ci], accs[2 + ci], tsl,
                                                evp, cos_sb, sin_sb, "ev")
                        rope_store(lo_o, hi_o, ci, krope_d, tsl)
                    if t == 0:
                        # kv_b weights for phase C ride behind the t=0 x tiles
                        kbw = kvwp.tile([P, KVRC, NH * DN], BF16)
                        vbw = kvwp.tile([P, KVRC, NH * DV], BF16)
                        nc.sync.dma_start(out=kbw[:], in_=kvbk_ap[:, :, :])
                        nc.sync.dma_start(out=vbw[:], in_=kvbv_ap[:, :, :])
                    if t == 1 and phmax >= 4:
                        # phase-C latents for t=0/1 prefetch during A2
                        kvn_pre[0] = load_lat(kvnp, "kvn", "kvn", "kv",
                                              KVRC, 0)
                        kvn_pre[1] = load_lat(kvnp, "kvn", "kvn", "kv",
                                              KVRC, 1)
            xpool2.release()
            krwp.release()
            xpool.release()

            # phase-B weights + t=0 latents prefetch (consumed after C)
            qnp = tc.alloc_tile_pool(name="qbn", bufs=2 * QRC + 1)
            qbwp = tc.alloc_tile_pool(name="qbw", bufs=1)
            qbw = qbwp.tile([P, QRC, NH * (DN + DR)], BF16)
            nc.sync.dma_start(out=qbw[:], in_=qb_ap[:, :, :])
            qn_pre = {}
            if phmax >= 3:
                qn_pre[0] = load_lat(qnp, "qn", "qn", "q", QRC, 0)

            # ------- Phase C: kv_b (k_nope + v) -----------------------------
            # emitted before q_b: it only needs the small kv gather
            with tc.tile_pool(name="kvev", bufs=4) as evp, \
                 tc.tile_pool(name="kvps", bufs=4, space="PSUM") as kvps:
                for t in range(ntt if phmax >= 4 else 0):
                    tsl = slice(t * TT, (t + 1) * TT)
                    kvn = kvn_pre.get(t) or load_lat(kvnp, "kvn", "kvn",
                                                     "kv", KVRC, t)
                    for m in range(NH):
                        ps = kvps.tile([P, TT], F32, tag="kps",
                                       name=f"kb{t}_{m}")
                        for k in range(KVRC):
                            nc.tensor.matmul(ps[:], kbw[:, k, m * P:(m + 1) * P],
                                             kvn[k][:], start=(k == 0),
                                             stop=(k == KVRC - 1))
                        o = evp.tile([P, TT], BF16, tag="ev")
                        nc.vector.tensor_copy(out=o[:], in_=ps[:])
                        nc.sync.dma_start(out=knope_d[:, m, tsl], in_=o[:])
                    for tc8 in range(TT // P):
                        for vc in range(NH * DV // TT):
                            ps = kvps.tile([P, TT], F32, tag="vps",
                                           name=f"v{t}_{tc8}_{vc}")
                            for k in range(KVRC):
                                nc.tensor.matmul(
                                    ps[:],
                                    kvn[k][:, tc8 * P:(tc8 + 1) * P],
                                    vbw[:, k, vc * TT:(vc + 1) * TT],
                                    start=(k == 0), stop=(k == KVRC - 1))
                            o = evp.tile([P, TT], BF16, tag="ev")
                            nc.vector.tensor_copy(out=o[:], in_=ps[:])
                            nc.sync.dma_start(
                                out=v_d[:, t * (TT // P) + tc8,
                                        vc * TT:(vc + 1) * TT],
                                in_=o[:])
            kvnp.release()
            kvwp.release()

            # o_proj weight pool reserved now (right side, outlives hp);
            # its DMA is emitted inside phase D, well ahead of phase E
            owp = tc.alloc_tile_pool(name="oww", bufs=1, side="right")
            oww = owp.tile([P, NH * DV // P, HID], BF16)
            hp = None  # allocated after B (SBUF peak there)
            nkt = tb // P  # key tiles

            def load_pair_k(j):
                """Attention key-side inputs for head pair j (ready at C-end)."""
                h0, h1 = 2 * j, 2 * j + 1
                kn0 = hp.tile([P, tb], BF16, tag="kn0", name=f"kn0_{j}")
                kn1 = hp.tile([P, tb], BF16, tag="kn1", name=f"kn1_{j}")
                nc.sync.dma_start(out=kn0[:], in_=knope_d[:, h0, :])
                nc.sync.dma_start(out=kn1[:], in_=knope_d[:, h1, :])
                krj = hp.tile([P, tb], BF16, tag="krj", name=f"krj{j}")
                rope_load(krj, krope_d, j)
                v0 = hp.tile([P, nkt, DV], BF16, tag="v0", name=f"v0_{j}")
                v1 = hp.tile([P, nkt, DV], BF16, tag="v1", name=f"v1_{j}")
                nc.sync.dma_start(out=v0[:],
                                  in_=v_d[:, :, h0 * DV:(h0 + 1) * DV])
                nc.sync.dma_start(out=v1[:],
                                  in_=v_d[:, :, h1 * DV:(h1 + 1) * DV])
                return kn0, kn1, krj, v0, v1

            def load_pair_q(j):
                h0, h1 = 2 * j, 2 * j + 1
                qn0 = hp.tile([P, tb], BF16, tag="qn0", name=f"qn0_{j}")
                qn1 = hp.tile([P, tb], BF16, tag="qn1", name=f"qn1_{j}")
                nc.sync.dma_start(out=qn0[:], in_=qnope_d[:, h0, :])
                nc.sync.dma_start(out=qn1[:], in_=qnope_d[:, h1, :])
                qrj = hp.tile([P, tb], BF16, tag="qrj", name=f"qrj{j}")
                rope_load(qrj, qrope_d, j)
                return qn0, qn1, qrj

            pair_pre = {}

            # ------- Phase B: q_b + q rope ----------------------------------
            with tc.tile_pool(name="qbev", bufs=10) as evp, \
                 tc.tile_pool(name="qbps", bufs=8, space="PSUM") as qbps:
                for t in range(ntt if phmax >= 3 else 0):
                    tsl = slice(t * TT, (t + 1) * TT)
                    qn = qn_pre.get(t) or load_lat(qnp, "qn", "qn", "q",
                                                   QRC, t)
                    rope_ps = {}
                    for m in range(QRC):
                        ps = qbps.tile([P, TT], F32, tag="qbps",
                                       name=f"qb{t}_{m}")
                        for k in range(QRC):
                            nc.tensor.matmul(ps[:], qbw[:, k, m * P:(m + 1) * P],
                                             qn[k][:], start=(k == 0),
                                             stop=(k == QRC - 1))
                        if m < NH:
                            o = evp.tile([P, TT], BF16, tag="ev")
                            nc.vector.tensor_copy(out=o[:], in_=ps[:])
                            nc.sync.dma_start(out=qnope_d[:, m, tsl], in_=o[:])
                        else:
                            rope_ps[m - NH] = ps
                    for ci in range(2):
                        lo_o, hi_o = rope_evict(rope_ps[ci], rope_ps[2 + ci],
                                                tsl, evp, cos_sb, sin_sb, "ev")
                        rope_store(lo_o, hi_o, ci, qrope_d, tsl)
            qbwp.release()
            qnp.release()
            ropep.release()
            # attention j=0 inputs: enqueue at B's tail so they load while
            # B's last tiles compute (knope/v ready since C, qnope at B end)
            hp = tc.alloc_tile_pool(name="ath", bufs=2, side="right")
            if phmax >= 5:
                pair_pre["k0"] = load_pair_k(0)
                pair_pre["q0"] = load_pair_q(0)

            # ------- Phase D: attention (head pairs, rope row-packed) -------
            with tc.tile_pool(name="atex", bufs=10) as exp_p, \
                 tc.tile_pool(name="atsum", bufs=6) as sump, \
                 tc.tile_pool(name="atev", bufs=6) as evp, \
                 tc.tile_pool(name="atsc", bufs=4, space="PSUM") as scp, \
                 tc.tile_pool(name="atpv", bufs=1, space="PSUM") as pvp:
                for j in range(NH // 2 if phmax >= 5 else 0):
                    h0, h1 = 2 * j, 2 * j + 1
                    if j == 0:
                        kn0, kn1, krj, v0, v1 = pair_pre["k0"]
                        qn0, qn1, qrj = pair_pre["q0"]
                    else:
                        kn0, kn1, krj, v0, v1 = load_pair_k(j)
                        qn0, qn1, qrj = load_pair_q(j)
                    if j == 0:
                        # o_proj weights for phase E load during attention
                        nc.sync.dma_start(out=oww[:], in_=ow_ap[:, :, :])
                    def mk_chain(qt):
                        """One query-tile attention chain; returns
                        (scores, pv, tail) closures over its own state."""
                        qsl = slice(qt * TT, (qt + 1) * TT)
                        sfx = qt % 2
                        pv0 = pvp.tile([P, TT], F32, tag=f"pv0{sfx}",
                                       name=f"pv0_{j}_{qt}")
                        pv1 = pvp.tile([P, TT], F32, tag=f"pv1{sfx}",
                                       name=f"pv1_{j}_{qt}")
                        # softmax denominator: bf16 pairwise tree over the exp
                        # tiles (binary counter of partials per head) - no
                        # serial chain, 2x DVE 16-bit mode
                        partials = ([], [])

                        def tree_push(hi, t):
                            lvl = 0
                            ps = partials[hi]
                            while ps and ps[-1][0] == lvl:
                                _, prev = ps.pop()
                                o = sump.tile([P, TT], BF16,
                                              tag=f"tr{hi}{sfx}",
                                              name=f"tr{hi}_{j}_{qt}_{lvl}")
                                nc.vector.tensor_add(out=o[:], in0=prev[:],
                                                     in1=t[:])
                                t = o
                                lvl += 1
                            ps.append((lvl, t))

                        def scores(kt):
                            ksl = slice(kt * P, (kt + 1) * P)
                            sc0 = scp.tile([P, TT], F32, tag="sc",
                                           name=f"sc0_{j}_{qt}_{kt}")
                            sc1 = scp.tile([P, TT], F32, tag="sc",
                                           name=f"sc1_{j}_{qt}_{kt}")
                            nc.tensor.matmul(sc0[:], kn0[:, ksl], qn0[:, qsl],
                                             start=True, stop=False)
                            nc.tensor.matmul(sc1[:], kn1[:, ksl], qn1[:, qsl],
                                             start=True, stop=False)
                            # K=64 rope matmuls: disjoint row groups run
                            # concurrently in the PE array (tile_position)
                            nc.tensor.matmul(sc0[:], krj[0:64, ksl],
                                             qrj[0:64, qsl],
                                             start=False, stop=True,
                                             tile_position=(0, 0))
                            nc.tensor.matmul(sc1[:], krj[64:128, ksl],
                                             qrj[64:128, qsl],
                                             start=False, stop=True,
                                             tile_position=(64, 0))
                            ex0 = exp_p.tile([P, TT], BF16, tag="ex",
                                             name=f"ex0_{j}_{qt}_{kt}")
                            ex1 = exp_p.tile([P, TT], BF16, tag="ex",
                                             name=f"ex1_{j}_{qt}_{kt}")
                            nc.scalar.activation(ex0[:], sc0[:], EXP_FN,
                                                 scale=SCALE)
                            nc.scalar.activation(ex1[:], sc1[:], EXP_FN,
                                                 scale=SCALE)
                            tree_push(0, ex0)
                            tree_push(1, ex1)
                            return ex0, ex1

                        def pv(kt, ex0, ex1):
                            st, sp = kt == 0, kt == nkt - 1
                            nc.tensor.matmul(pv0[:], v0[:, kt, :], ex0[:],
                                             start=st, stop=sp)
                            nc.tensor.matmul(pv1[:], v1[:, kt, :], ex1[:],
                                             start=st, stop=sp)

                        def tail():
                            for hi, (h, pvt) in enumerate(((h0, pv0),
                                                           (h1, pv1))):
                                # drain the tree (nkt power of two -> 1 entry)
                                ps = partials[hi]
                                while len(ps) > 1:
                                    _, a = ps.pop()
                                    _, b = ps.pop()
                                    o = sump.tile([P, TT], BF16,
                                                  tag=f"tr{hi}{sfx}",
                                                  name=f"trd{hi}_{j}_{qt}")
                                    nc.vector.tensor_add(out=o[:], in0=a[:],
                                                         in1=b[:])
                                    ps.append((99, o))
                                es = ps.pop()[1]
                                den = scp.tile([P, TT], F32, tag="sc",
                                               name=f"den_{j}_{qt}_{h}")
                                nc.tensor.matmul(den[:], ones_b[:], es[:],
                                                 start=True, stop=True)
                                recip = evp.tile([P, TT], F32, tag="evr",
                                                 name="recip")
                                # den in [~1, ~3e3]: approx_fast's 18 bits
                                # are far below bf16 noise, ~5x cheaper
                                nc.vector.reciprocal_approx_fast(recip[:],
                                                                 den[:])
                                ao = evp.tile([P, TT], BF16, tag="ev",
                                              name="ao")
                                nc.vector.tensor_mul(out=ao[:], in0=pvt[:],
                                                     in1=recip[:])
                                nc.sync.dma_start(out=attn_d[:, h, qsl],
                                                  in_=ao[:])
                        return scores, pv, tail

                    # two query-tile chains interleaved per head pair: the PE
                    # always has the other chain's scores between a chain's
                    # exp and its pv, so it never waits on the ACT engine
                    qts = list(range(ntt))
                    while qts:
                        if len(qts) >= 2:
                            sa, pa, ta = mk_chain(qts.pop(0))
                            sb, pb, tb_ = mk_chain(qts.pop(0))
                            exa = [sa(0)]
                            exb = [sb(0)]
                            for kt in range(1, nkt):
                                exa.append(sa(kt))
                                pa(kt - 1, *exa.pop(0))
                                exb.append(sb(kt))
                                pb(kt - 1, *exb.pop(0))
                            pa(nkt - 1, *exa.pop(0))
                            pb(nkt - 1, *exb.pop(0))
                            ta()
                            tb_()
                        else:
                            sa, pa, ta = mk_chain(qts.pop(0))
                            exa = [sa(0), sa(1)]
                            for kt in range(2, nkt):
                                exa.append(sa(kt))
                                pa(kt - 2, *exa.pop(0))
                            pa(nkt - 2, *exa.pop(0))
                            pa(nkt - 1, *exa.pop(0))
                            ta()

            hp.release()

            # ------- Phase E: o_proj (partial) ------------------------------
            with tc.tile_pool(name="oin", bufs=4) as inp, \
                 tc.tile_pool(name="oev", bufs=4) as evp, \
                 tc.tile_pool(name="ops", bufs=6, space="PSUM") as ops:
                for t8 in range(ntc if phmax >= 6 else 0):
                    at = inp.tile([P, NH, P], BF16, tag="at", name=f"at{t8}")
                    nc.sync.dma_start(out=at[:],
                                      in_=attn_d[:, :, t8 * P:(t8 + 1) * P])
                    for n in range(HID // TT):
                        ps = ops.tile([P, TT], F32, tag="ops", name=f"o{t8}_{n}")
                        for k in range(NH * DV // P):
                            nc.tensor.matmul(ps[:], at[:, k, :],
                                             oww[:, k, n * TT:(n + 1) * TT],
                                             start=(k == 0),
                                             stop=(k == NH * DV // P - 1))
                        o = evp.tile([P, TT], BF16, tag="ev")
                        nc.vector.tensor_copy(out=o[:], in_=ps[:])
                        nc.sync.dma_start(
                            out=out_part[t8 * P:(t8 + 1) * P,
                                         n * TT:(n + 1) * TT],
                            in_=o[:])
            owp.release()

    nc.compile()
    return nc


# ---------------------------------------------------------------------------
# host-side packing
# ---------------------------------------------------------------------------

def _rope_tables():
    inv_freq = 1.0 / (10000.0 ** (np.arange(0, DR, 2, dtype=np.float32) / DR))
    t = np.arange(MAXP, dtype=np.float32)
    freqs = np.outer(t, inv_freq)
    emb = np.concatenate([freqs, freqs], axis=-1)
    return np.cos(emb).astype(np.float32), np.sin(emb).astype(np.float32)


def core_weights(g, q_a_w, q_a_ln_w, q_b_w, kv_a_w, kv_a_ln_w, kv_b_w,
                 k_rope_w, o_w):
    """Pack the weight set for head-group g (heads g*8 .. g*8+8)."""
    heads = range(g * NH, (g + 1) * NH)
    qb_eff = (q_b_w * q_a_ln_w[None, :]).astype(np.float32)
    kvb_eff = (kv_b_w * kv_a_ln_w[None, :]).astype(np.float32)

    nope_rows = np.concatenate(
        [np.arange(h * (DN + DR), h * (DN + DR) + DN) for h in heads])
    lo_rows = np.concatenate(
        [np.arange(h * (DN + DR) + DN, h * (DN + DR) + DN + 32) for h in heads])
    hi_rows = np.concatenate(
        [np.arange(h * (DN + DR) + DN + 32, h * (DN + DR) + DN + 64)
         for h in heads])
    qb_rows = np.concatenate([nope_rows, lo_rows, hi_rows])

    k_rows = np.concatenate(
        [np.arange(h * (DN + DV), h * (DN + DV) + DN) for h in heads])
    v_rows = np.concatenate(
        [np.arange(h * (DN + DV) + DN, (h + 1) * (DN + DV)) for h in heads])

    kr_lo = np.concatenate([np.arange(h * DR, h * DR + 32) for h in heads])
    kr_hi = np.concatenate([np.arange(h * DR + 32, (h + 1) * DR) for h in heads])
    kr_rows = np.concatenate([kr_lo, kr_hi])

    o_cols = np.concatenate([np.arange(h * DV, (h + 1) * DV) for h in heads])

    def c(a):
        return np.ascontiguousarray(a).astype(NP_BF16)

    return {
        "qa_wT": c(q_a_w.T),
        "kva_wT": c(kv_a_w.T),
        "kr_wT": c(k_rope_w[kr_rows].T),
        "qb_wT": c(qb_eff[qb_rows].T),
        "kvbk_wT": c(kvb_eff[k_rows].T),
        "kvbv_wT": c(kvb_eff[v_rows].T),
        "o_wT": c(o_w[:, o_cols].T),
    }


def core_inputs(c, hidden_states, position_ids, weight_sets, tb=S,
                seqpar=SEQPAR):
    b, g = c // 4, c % 4
    tta = tb // 4
    cos_tab, sin_tab = _rope_tables()
    pos = np.asarray(position_ids[b][:tb])
    cos_b = cos_tab[pos]  # [tb, DR]
    sin_b = sin_tab[pos]
    cos_rep = np.ascontiguousarray(np.tile(cos_b[:, :32].T, (4, 1))).astype(
        NP_BF16)
    sin_rep = np.ascontiguousarray(np.tile(sin_b[:, :32].T, (4, 1))).astype(
        NP_BF16)
    x = np.asarray(hidden_states[b][:tb], dtype=np.float32)
    xTb = np.ascontiguousarray(x.T).astype(NP_BF16)
    im = {"xT": xTb, "cos_rep": cos_rep, "sin_rep": sin_rep}
    if seqpar:
        im["xA"] = np.ascontiguousarray(xTb[:, g * tta:(g + 1) * tta])
    im.update(weight_sets[g])
    return im


_CACHE = {}


def _get_nc(tb=S):
    if tb not in _CACHE:
        _CACHE[tb] = build_nc(tb)
    return _CACHE[tb]


def kernel(hidden_states, position_ids, q_a_w, q_a_ln_w, q_b_w,
           kv_a_w, kv_a_ln_w, kv_b_w, k_rope_w, o_w):
    hidden_states = np.asarray(hidden_states, dtype=np.float32)
    weight_sets = [
        core_weights(g, np.asarray(q_a_w, np.float32),
                     np.asarray(q_a_ln_w, np.float32),
                     np.asarray(q_b_w, np.float32),
                     np.asarray(kv_a_w, np.float32),
                     np.asarray(kv_a_ln_w, np.float32),
                     np.asarray(kv_b_w, np.float32),
                     np.asarray(k_rope_w, np.float32),
                     np.asarray(o_w, np.float32))
        for g in range(4)
    ]
    in_maps = [core_inputs(c, hidden_states, position_ids, weight_sets)
               for c in range(NCORES)]
    nc = _get_nc()
    res = run_bass_kernel_spmd(nc, in_maps, core_ids=list(range(NCORES)))
    out = np.zeros((B, S, HID), dtype=np.float64)
    for c in range(NCORES):
        out[c // 4] += np.asarray(res.results[c]["out_part"], np.float32)
    return out.astype(np.float32)


# revision 40
# speedup vs baseline: 1.0599x; 1.0599x over previous
"""DeepSeekV3 MLA attention kernel for Trainium2 (8 NeuronCores, Bass/Tile).

Sharding: core c -> batch b = c // 4, head-group g = c % 4 (8 of 32 heads).
Each core runs the layer for its batch restricted to its heads and emits a
partial o_proj output [2048, 4096] (bf16); the host sums the 4 partials per
batch in f32.

The shared a-projections (q_a, kv_a) are *sequence-parallel*: each core
computes and RMS-normalizes the latents for its 512-token slice only, then
AllGathers over the 4-core batch group replicate the full normalized latents.
The kv gather fires first (small, unblocks kv_b early); the q gather is split
in two so wire time pipelines with the tail of the q_a compute.  A tiny dummy
AllGather is issued before any compute to absorb the one-time collective
entry barrier (~47us) underneath phase-A compute.

Layouts (feature-major, [128, chunks, tokens]); all matmul operands bf16
(full PE rate), PSUM accumulation fp32:
  - x is fed transposed (xT [4096, 2048]); matmuls contract over the
    partition dim with N = 512 token tiles (one PSUM bank).
  - RoPE halves are packed [4*lo(32) | 4*hi(32)] per 4 heads; the rotated
    output is stored to DRAM in its natural [lo/hi, ci, 128, tb] layout and
    attention re-gathers the 4 32-row groups per head pair with four block
    DMAs (replaces the former 16-way scatter-DMA storm per tile).
    Head pairs (2j, 2j+1) land in partition halves [0:64], [64:128] of the
    attention rope tile, so attention row-packs the two K=64 rope matmuls
    into one PE pass via tile_position (concurrent sub-array execution).
  - Softmax skips the max-subtraction (scores are O(5), exp safe in fp32).
    Denominators: exp tiles are summed with a bf16 pairwise tree on the DVE
    (binary counter of partials, no serial chain, 2x 16-bit mode) and
    reduced with a single ones-matmul per (head, query-tile) - removes the
    former per-key-tile ones matmuls (~8.6 GFLOP of PE time).  1/den uses
    reciprocal_approx_fast (~5x cheaper, accuracy far below bf16 noise).
    The score->exp->pv chain is software-pipelined two key-tiles deep so
    the PE never waits on the ACT exp (keeps the HAM clock-gate warm).
  - Phase weights (kv_b, q_b, o_w) are prefetched one phase ahead, phase
    input streams (A2's x tiles, C/B latents, attention j=0 tiles) are
    enqueued before the previous phase's data-dependent stores so no phase
    opens behind a head-of-line-blocked DMA queue.
"""

import math

import numpy as np

try:
    import concourse.bacc as bacc  # noqa: F401
except ImportError:
    import sys

    for _p in ("/root/.axon_site/_ro/trn_rl_repo", "/opt/trn_rl_repo"):
        if _p not in sys.path:
            sys.path.insert(0, _p)

import concourse.bacc as bacc
import concourse.mybir as mybir
import concourse.tile as tile
from concourse.bass_utils import run_bass_kernel_spmd

# model dims
H, DN, DR, DV = 32, 128, 64, 128
HID, QR, KVR = 4096, 1536, 512
EPS, MAXP = 1e-6, 4096
B, S = 2, 2048
P = 128
TT = 512  # token tile (matmul moving dim)
NH = 8  # heads per core
NCORES = 8
SCALE = 1.0 / math.sqrt(DN + DR)
HIDC = HID // P  # 32
QRC = QR // P  # 12
KVRC = KVR // P  # 4

F32 = mybir.dt.float32
BF16 = mybir.dt.bfloat16
NP_BF16 = mybir.dt.np(BF16)

EXP_FN = mybir.ActivationFunctionType.Exp
SQRT_FN = mybir.ActivationFunctionType.Sqrt

SEQPAR = True  # sequence-parallel a-projections via AllGather
QSPLIT = 6  # q latent chunks in the first of the two q AllGathers


def build_nc(tb=S, seqpar=SEQPAR):
    """Build the per-core Bass program (same program on all 8 cores)."""
    import os as _os
    phmax = int(_os.environ.get("PHMAX", "9"))
    ntt = tb // TT  # token tiles for phases B..E
    ntc = tb // P  # token chunks
    tta = tb // 4  # a-projection slice length per core
    nc = bacc.Bacc("TRN2", target_bir_lowering=False, debug=False,
                   num_devices=NCORES)

    xT = nc.dram_tensor("xT", [HID, tb], BF16, kind="ExternalInput")
    qa_wT = nc.dram_tensor("qa_wT", [HID, QR], BF16, kind="ExternalInput")
    kva_wT = nc.dram_tensor("kva_wT", [HID, KVR], BF16, kind="ExternalInput")
    kr_wT = nc.dram_tensor("kr_wT", [HID, NH * DR], BF16, kind="ExternalInput")
    qb_wT = nc.dram_tensor("qb_wT", [QR, NH * (DN + DR)], BF16,
                           kind="ExternalInput")
    kvbk_wT = nc.dram_tensor("kvbk_wT", [KVR, NH * DN], BF16,
                             kind="ExternalInput")
    kvbv_wT = nc.dram_tensor("kvbv_wT", [KVR, NH * DV], BF16,
                             kind="ExternalInput")
    o_wT = nc.dram_tensor("o_wT", [NH * DV, HID], BF16, kind="ExternalInput")
    cos_in = nc.dram_tensor("cos_rep", [P, tb], BF16, kind="ExternalInput")
    sin_in = nc.dram_tensor("sin_rep", [P, tb], BF16, kind="ExternalInput")
    if seqpar:
        xA = nc.dram_tensor("xA", [HID, tta], BF16, kind="ExternalInput")
    out_part = nc.dram_tensor("out_part", [tb, HID], BF16,
                              kind="ExternalOutput")

    x_ap = xT[:, :].rearrange("(c p) t -> p c t", p=P)
    qa_ap = qa_wT[:, :].rearrange("(c p) m -> p c m", p=P)
    kva_ap = kva_wT[:, :].rearrange("(c p) m -> p c m", p=P)
    kr_ap = kr_wT[:, :].rearrange("(c p) m -> p c m", p=P)
    qb_ap = qb_wT[:, :].rearrange("(c p) m -> p c m", p=P)
    kvbk_ap = kvbk_wT[:, :].rearrange("(c p) m -> p c m", p=P)
    kvbv_ap = kvbv_wT[:, :].rearrange("(c p) m -> p c m", p=P)
    ow_ap = o_wT[:, :].rearrange("(c p) m -> p c m", p=P)
    if seqpar:
        xa_ap = xA[:, :].rearrange("(c p) t -> p c t", p=P)

    with tile.TileContext(nc) as tc:
        with tc.tile_pool(name="const", bufs=1) as constp, \
             tc.tile_pool(name="dram", bufs=1, space="DRAM") as dram:
            ones_f = constp.tile([P, P], F32)
            nc.any.memset(ones_f[:], 1.0)
            ones_b = constp.tile([P, P], BF16)
            nc.vector.tensor_copy(out=ones_b[:], in_=ones_f[:])
            eps_sb = constp.tile([P, 1], F32)
            nc.any.memset(eps_sb[:], EPS)

            # dummy collective: absorbs the one-time CC entry barrier under
            # phase-A compute (the real gathers then start immediately).
            if seqpar:
                dummy_in = dram.tile([P, 1], BF16)
                dummy_out = dram.tile([4 * P, 1], BF16)
                nc.gpsimd.collective_compute(
                    "AllGather", mybir.AluOpType.bypass,
                    replica_groups=[[0, 1, 2, 3], [4, 5, 6, 7]],
                    ins=[dummy_in.opt()], outs=[dummy_out.opt()])

            # gathered normalized latents: block g = tokens [g*tta,(g+1)*tta)
            latq_in = dram.tile([QRC, P, tta], BF16)
            latq_all_a = dram.tile([4 * QSPLIT, P, tta], BF16)
            latq_all_b = dram.tile([4 * (QRC - QSPLIT), P, tta], BF16)
            latkv_in = dram.tile([KVRC, P, tta], BF16)
            latkv_all = dram.tile([4 * KVRC, P, tta], BF16)
            qnope_d = dram.tile([P, NH, tb], BF16)
            knope_d = dram.tile([P, NH, tb], BF16)
            # rope in natural producer layout: [lo/hi, ci, 128, tb]
            qrope_d = dram.tile([2, 2, P, tb], BF16)
            krope_d = dram.tile([2, 2, P, tb], BF16)
            v_d = dram.tile([P, ntc, NH * DV], BF16)
            attn_d = dram.tile([P, NH, tb], BF16)

            def rope_evict(lo_src, hi_src, tsl, pool, cos_sb, sin_sb, tag):
                """lo/hi chunk pair [P, n] (4 heads x 32 rows) -> rotate."""
                t1 = pool.tile([P, TT], F32, tag=tag, name="rt1")
                t2 = pool.tile([P, TT], F32, tag=tag, name="rt2")
                n = tsl.stop - tsl.start
                nc.vector.tensor_mul(out=t1[:, :n], in0=lo_src[:],
                                     in1=cos_sb[:, tsl])
                nc.vector.tensor_mul(out=t2[:, :n], in0=hi_src[:],
                                     in1=sin_sb[:, tsl])
                lo_o = pool.tile([P, TT], BF16, tag=tag, name="rlo")
                nc.vector.tensor_sub(out=lo_o[:, :n], in0=t1[:, :n],
                                     in1=t2[:, :n])
                t3 = pool.tile([P, TT], F32, tag=tag, name="rt3")
                t4 = pool.tile([P, TT], F32, tag=tag, name="rt4")
                nc.vector.tensor_mul(out=t3[:, :n], in0=hi_src[:],
                                     in1=cos_sb[:, tsl])
                nc.vector.tensor_mul(out=t4[:, :n], in0=lo_src[:],
                                     in1=sin_sb[:, tsl])
                hi_o = pool.tile([P, TT], BF16, tag=tag, name="rhi")
                nc.vector.tensor_add(out=hi_o[:, :n], in0=t3[:, :n],
                                     in1=t4[:, :n])
                return lo_o, hi_o

            def rope_store(lo_o, hi_o, ci, dst_d, tsl):
                n = tsl.stop - tsl.start
                nc.sync.dma_start(out=dst_d[0, ci, :, tsl], in_=lo_o[:, :n])
                nc.sync.dma_start(out=dst_d[1, ci, :, tsl], in_=hi_o[:, :n])

            def rope_load(dst, src_d, j):
                """Gather pair j's rope rows [e-lo, e-hi, o-lo, o-hi]."""
                ci, hh0 = j // 2, (j % 2) * 2
                for r, (lh, hh) in enumerate(
                        ((0, hh0), (1, hh0), (0, hh0 + 1), (1, hh0 + 1))):
                    nc.sync.dma_start(
                        out=dst[32 * r:32 * r + 32, :],
                        in_=src_d[lh, ci, 32 * hh:32 * hh + 32, :])

            # pools for the A2 input stream, allocated under the A1 pools so
            # their DMAs can be emitted inside A1 (ahead of the q-latent
            # stores, which only execute once A1's PE stream finishes --
            # emitting A2's loads after them would head-of-line block A2)
            ropep = tc.alloc_tile_pool(name="ropetab", bufs=1)
            xpool = tc.alloc_tile_pool(name="krx", bufs=33)
            krwp = tc.alloc_tile_pool(name="krw", bufs=4)
            xt_pre, krw_tiles, rope_tabs = [], [], []

            def emit_a2_prefetch():
                for k in range(HIDC):
                    xt = xpool.tile([P, TT], BF16, tag="krx",
                                    name=f"krx0_{k}")
                    nc.sync.dma_start(out=xt[:], in_=x_ap[:, k, 0:TT])
                    xt_pre.append(xt)
                cos_sb = ropep.tile([P, tb], BF16)
                sin_sb = ropep.tile([P, tb], BF16)
                nc.sync.dma_start(out=cos_sb[:], in_=cos_in[:, :])
                nc.sync.dma_start(out=sin_sb[:], in_=sin_in[:, :])
                rope_tabs.extend([cos_sb, sin_sb])
                for m in range(4):
                    wt = krwp.tile([P, HIDC, P], BF16, tag="krw",
                                   name=f"krw{m}")
                    for pt in range(4):
                        ks = slice(8 * pt, 8 * (pt + 1))
                        nc.sync.dma_start(out=wt[:, ks, :],
                                          in_=kr_ap[:, ks, m * P:(m + 1) * P])
                    krw_tiles.append(wt)

            # ------- Phase A1: seq-sliced q_a/kv_a + rms-norm + gather -------
            # groups of output chunks; contraction over HID (32 k-chunks)
            with tc.tile_pool(name="apw", bufs=7) as wpool, \
                 tc.tile_pool(name="apx", bufs=1 if seqpar else 2) as axp, \
                 tc.tile_pool(name="apraw", bufs=QRC + KVRC + 1) as rawp, \
                 tc.tile_pool(name="apev", bufs=8) as evp, \
                 tc.tile_pool(name="apacc", bufs=6, space="PSUM") as accp, \
                 tc.tile_pool(name="apstat", bufs=2, space="PSUM") as statp:
                slices = range(1) if seqpar else range(4)
                if phmax < 1:
                    slices = range(0)

                def norm_and_ship(kind, raws, stats, g4, m0, m1):
                    rank = QR if kind == "q" else KVR
                    sdev = evp.tile([P, tta], F32, tag="ev")
                    nc.scalar.activation(sdev[:], stats[kind][:], SQRT_FN,
                                         bias=eps_sb[:], scale=1.0 / rank)
                    rstd = evp.tile([P, tta], F32, tag="ev")
                    nc.vector.reciprocal(rstd[:], sdev[:])
                    for m in range(m0, m1):
                        nrm = evp.tile([P, tta], BF16, tag="ev")
                        nc.vector.tensor_mul(out=nrm[:],
                                             in0=raws[(kind, m)][:],
                                             in1=rstd[:])
                        if seqpar:
                            l_in = latq_in if kind == "q" else latkv_in
                            nc.sync.dma_start(out=l_in[m], in_=nrm[:])
                        else:
                            if kind == "kv":
                                nc.sync.dma_start(
                                    out=latkv_all[g4 * KVRC + m], in_=nrm[:])
                            elif m < QSPLIT:
                                nc.sync.dma_start(
                                    out=latq_all_a[g4 * QSPLIT + m],
                                    in_=nrm[:])
                            else:
                                nc.sync.dma_start(
                                    out=latq_all_b[
                                        g4 * (QRC - QSPLIT) + m - QSPLIT],
                                    in_=nrm[:])

                def gather(ins, outs):
                    nc.gpsimd.collective_compute(
                        "AllGather", mybir.AluOpType.bypass,
                        replica_groups=[[0, 1, 2, 3], [4, 5, 6, 7]],
                        ins=[ins.opt()], outs=[outs.opt()])

                for g4 in slices:
                    asl = slice(g4 * tta, (g4 + 1) * tta)
                    xs = axp.tile([P, HIDC, tta], BF16, tag="apx",
                                    name=f"xs{g4}")

                    def xs_load(k):
                        if seqpar:
                            nc.sync.dma_start(out=xs[:, k, :],
                                              in_=xa_ap[:, k, :])
                        else:
                            nc.sync.dma_start(out=xs[:, k, :],
                                              in_=x_ap[:, k, asl])

                    # stage the first k-chunks + first weights in small DMAs
                    # so the opening matmul isn't behind a multi-MB queue
                    for k in range(8):
                        xs_load(k)
                    first_group = True
                    raws = {}
                    stats = {}
                    # rms-norm stat accumulated per kind across all m-groups
                    for kind, m0, m1 in [("kv", 0, KVRC), ("q", 0, QSPLIT),
                                         ("q", QSPLIT, QRC)]:
                        src = qa_ap if kind == "q" else kva_ap
                        nm = QRC if kind == "q" else KVRC
                        wts = []
                        for m in range(m0, m1):
                            wt = wpool.tile([P, HIDC, P], BF16, tag="apw",
                                            name=f"apw{kind}{g4}_{m}")
                            if first_group:
                                for pt in range(4):
                                    ks = slice(8 * pt, 8 * (pt + 1))
                                    nc.sync.dma_start(
                                        out=wt[:, ks, :],
                                        in_=src[:, ks, m * P:(m + 1) * P])
                            else:
                                nc.sync.dma_start(
                                    out=wt[:],
                                    in_=src[:, :, m * P:(m + 1) * P])
                            wts.append(wt)
                        if first_group:
                            for k in range(8, HIDC):
                                xs_load(k)
                            first_group = False
                        accs = [accp.tile([P, tta], F32, tag="acc",
                                          name=f"acc{kind}{g4}_{m}")
                                for m in range(m0, m1)]
                        for k in range(HIDC):
                            for mi in range(m1 - m0):
                                nc.tensor.matmul(
                                    accs[mi][:], wts[mi][:, k, :], xs[:, k, :],
                                    start=(k == 0), stop=(k == HIDC - 1))
                        if kind not in stats:
                            stats[kind] = statp.tile(
                                [P, tta], F32, tag="stat",
                                name=f"stat{kind}_{g4}")
                        stat = stats[kind]
                        for mi, m in enumerate(range(m0, m1)):
                            raw = rawp.tile([P, tta], BF16, tag="raw",
                                            name=f"raw{kind}{g4}_{m}")
                            nc.vector.tensor_copy(out=raw[:], in_=accs[mi][:])
                            raws[(kind, m)] = raw
                            sq = evp.tile([P, tta], BF16, tag="ev")
                            nc.vector.tensor_mul(out=sq[:], in0=raw[:],
                                                 in1=raw[:])
                            nc.tensor.matmul(stat[:], ones_b[:], sq[:],
                                             start=(m == 0), stop=(m == nm - 1))
                        if kind == "kv":
                            norm_and_ship("kv", raws, stats, g4, 0, KVRC)
                            if seqpar and phmax >= 1 and g4 == slices[-1]:
                                gather(latkv_in, latkv_all)
                        elif m1 == QRC:
                            # A2's input stream enqueues here: after all A1
                            # weight loads, before the q-latent stores
                            if not xt_pre:
                                emit_a2_prefetch()
                            # second q half: stat now complete -> norm all q
                            norm_and_ship("q", raws, stats, g4, 0, QSPLIT)
                            if seqpar and phmax >= 1 and g4 == slices[-1]:
                                gather(latq_in[0:QSPLIT], latq_all_a)
                            norm_and_ship("q", raws, stats, g4, QSPLIT, QRC)
                            if seqpar and phmax >= 1 and g4 == slices[-1]:
                                gather(latq_in[QSPLIT:QRC], latq_all_b)

            def load_lat(pool, tag, name, kind, nk, t):
                """Load latent chunks [0, nk) for token tile t."""
                tiles = []
                for kk in range(nk):
                    qn = pool.tile([P, TT], BF16, tag=tag,
                                   name=f"{name}{t}_{kk}")
                    if kind == "q":
                        nm_a, nm_b = QSPLIT, QRC - QSPLIT
                    for bk in range(4):
                        lo, hi = bk * tta, (bk + 1) * tta
                        if lo >= t * TT and hi <= (t + 1) * TT:
                            if kind == "kv":
                                src = latkv_all[bk * KVRC + kk]
                            elif kk < QSPLIT:
                                src = latq_all_a[bk * nm_a + kk]
                            else:
                                src = latq_all_b[bk * nm_b + kk - QSPLIT]
                            nc.sync.dma_start(
                                out=qn[:, lo - t * TT:hi - t * TT], in_=src)
                    tiles.append(qn)
                return tiles

            # ------- Phase A2: k_rope (all tb tokens, this core's heads) -----
            if not xt_pre:  # phmax debug path: A1 skipped entirely
                emit_a2_prefetch()
            cos_sb, sin_sb = rope_tabs
            wts = krw_tiles
            # second x pool in the space A1 just freed: gives the t>=1 x
            # stream real DMA lookahead (one full tile + rotation)
            xpool2 = tc.alloc_tile_pool(name="krx2", bufs=36)
            xt_pre2 = []
            if ntt > 1 and phmax >= 2:
                for k in range(HIDC):
                    xt = xpool2.tile([P, TT], BF16, tag="krx2",
                                     name=f"krx1_{k}")
                    nc.sync.dma_start(out=xt[:], in_=x_ap[:, k, TT:2 * TT])
                    xt_pre2.append(xt)
            kvwp = tc.alloc_tile_pool(name="kvw", bufs=1, side="right")
            kvnp = tc.alloc_tile_pool(name="kvn", bufs=2 * KVRC + 1,
                                      side="right")

            with tc.tile_pool(name="krev", bufs=10) as evp, \
                 tc.tile_pool(name="kracc", bufs=5, space="PSUM") as accp:
                kvn_pre = {}
                for t in range(ntt if phmax >= 2 else 0):
                    tsl = slice(t * TT, (t + 1) * TT)
                    accs = [accp.tile([P, TT], F32, tag="acc",
                                      name=f"kracc{t}_{m}") for m in range(4)]
                    for k in range(HIDC):
                        if t == 0:
                            xt = xt_pre[k]
                        elif t == 1:
                            xt = xt_pre2[k]
                        else:
                            xt = xpool2.tile([P, TT], BF16, tag="krx2",
                                             name=f"krx{t}_{k}")
                            nc.sync.dma_start(out=xt[:], in_=x_ap[:, k, tsl])
                        for mi in range(4):
                            nc.tensor.matmul(
                                accs[mi][:], wts[mi][:, k, :], xt[:],
                                start=(k == 0), stop=(k == HIDC - 1))
                    # chunks [lo0, lo1, hi0, hi1] -> rope
                    for ci in range(2):
                        lo_o, hi_o = rope_evict(accs[ci], accs[2 + ci], tsl,
                                                evp, cos_sb, sin_sb, "ev")
                        rope_store(lo_o, hi_o, ci, krope_d, tsl)
                    if t == 0:
                        # kv_b weights for phase C ride behind the t=0 x tiles
                        kbw = kvwp.tile([P, KVRC, NH * DN], BF16)
                        vbw = kvwp.tile([P, KVRC, NH * DV], BF16)
                        nc.sync.dma_start(out=kbw[:], in_=kvbk_ap[:, :, :])
                        nc.sync.dma_start(out=vbw[:], in_=kvbv_ap[:, :, :])
                    if t == 1 and phmax >= 4:
                        # phase-C latents for t=0/1 prefetch during A2
                        kvn_pre[0] = load_lat(kvnp, "kvn", "kvn", "kv",
                                              KVRC, 0)
                        kvn_pre[1] = load_lat(kvnp, "kvn", "kvn", "kv",
                                              KVRC, 1)
            xpool2.release()
            krwp.release()
            xpool.release()

            # phase-B weights + t=0 latents prefetch (consumed after C)
            qnp = tc.alloc_tile_pool(name="qbn", bufs=2 * QRC + 1)
            qbwp = tc.alloc_tile_pool(name="qbw", bufs=1)
            qbw = qbwp.tile([P, QRC, NH * (DN + DR)], BF16)
            nc.sync.dma_start(out=qbw[:], in_=qb_ap[:, :, :])
            qn_pre = {}
            if phmax >= 3:
                qn_pre[0] = load_lat(qnp, "qn", "qn", "q", QRC, 0)

            # ------- Phase C: kv_b (k_nope + v) -----------------------------
            # emitted before q_b: it only needs the small kv gather
            with tc.tile_pool(name="kvev", bufs=4) as evp, \
                 tc.tile_pool(name="kvps", bufs=4, space="PSUM") as kvps:
                for t in range(ntt if phmax >= 4 else 0):
                    tsl = slice(t * TT, (t + 1) * TT)
                    kvn = kvn_pre.get(t) or load_lat(kvnp, "kvn", "kvn",
                                                     "kv", KVRC, t)
                    for m in range(NH):
                        ps = kvps.tile([P, TT], F32, tag="kps",
                                       name=f"kb{t}_{m}")
                        for k in range(KVRC):
                            nc.tensor.matmul(ps[:], kbw[:, k, m * P:(m + 1) * P],
                                             kvn[k][:], start=(k == 0),
                                             stop=(k == KVRC - 1))
                        o = evp.tile([P, TT], BF16, tag="ev")
                        nc.vector.tensor_copy(out=o[:], in_=ps[:])
                        nc.sync.dma_start(out=knope_d[:, m, tsl], in_=o[:])
                    for tc8 in range(TT // P):
                        for vc in range(NH * DV // TT):
                            ps = kvps.tile([P, TT], F32, tag="vps",
                                           name=f"v{t}_{tc8}_{vc}")
                            for k in range(KVRC):
                                nc.tensor.matmul(
                                    ps[:],
                                    kvn[k][:, tc8 * P:(tc8 + 1) * P],
                                    vbw[:, k, vc * TT:(vc + 1) * TT],
                                    start=(k == 0), stop=(k == KVRC - 1))
                            o = evp.tile([P, TT], BF16, tag="ev")
                            nc.vector.tensor_copy(out=o[:], in_=ps[:])
                            nc.sync.dma_start(
                                out=v_d[:, t * (TT // P) + tc8,
                                        vc * TT:(vc + 1) * TT],
                                in_=o[:])
            kvnp.release()
            kvwp.release()

            # o_proj weight pool reserved now (right side, outlives hp);
            # its DMA is emitted inside phase D, well ahead of phase E
            owp = tc.alloc_tile_pool(name="oww", bufs=1, side="right")
            oww = owp.tile([P, NH * DV // P, HID], BF16)
            hp = None  # allocated after B (SBUF peak there)
            nkt = tb // P  # key tiles

            def load_pair_k(j):
                """Attention key-side inputs for head pair j (ready at C-end)."""
                h0, h1 = 2 * j, 2 * j + 1
                kn0 = hp.tile([P, tb], BF16, tag="kn0", name=f"kn0_{j}")
                kn1 = hp.tile([P, tb], BF16, tag="kn1", name=f"kn1_{j}")
                nc.sync.dma_start(out=kn0[:], in_=knope_d[:, h0, :])
                nc.sync.dma_start(out=kn1[:], in_=knope_d[:, h1, :])
                krj = hp.tile([P, tb], BF16, tag="krj", name=f"krj{j}")
                rope_load(krj, krope_d, j)
                v0 = hp.tile([P, nkt, DV], BF16, tag="v0", name=f"v0_{j}")
                v1 = hp.tile([P, nkt, DV], BF16, tag="v1", name=f"v1_{j}")
                nc.sync.dma_start(out=v0[:],
                                  in_=v_d[:, :, h0 * DV:(h0 + 1) * DV])
                nc.sync.dma_start(out=v1[:],
                                  in_=v_d[:, :, h1 * DV:(h1 + 1) * DV])
                return kn0, kn1, krj, v0, v1

            def load_pair_q(j):
                h0, h1 = 2 * j, 2 * j + 1
                qn0 = hp.tile([P, tb], BF16, tag="qn0", name=f"qn0_{j}")
                qn1 = hp.tile([P, tb], BF16, tag="qn1", name=f"qn1_{j}")
                nc.sync.dma_start(out=qn0[:], in_=qnope_d[:, h0, :])
                nc.sync.dma_start(out=qn1[:], in_=qnope_d[:, h1, :])
                qrj = hp.tile([P, tb], BF16, tag="qrj", name=f"qrj{j}")
                rope_load(qrj, qrope_d, j)
                return qn0, qn1, qrj

            pair_pre = {}

            # ------- Phase B: q_b + q rope ----------------------------------
            with tc.tile_pool(name="qbev", bufs=10) as evp, \
                 tc.tile_pool(name="qbps", bufs=8, space="PSUM") as qbps:
                for t in range(ntt if phmax >= 3 else 0):
                    tsl = slice(t * TT, (t + 1) * TT)
                    qn = qn_pre.get(t) or load_lat(qnp, "qn", "qn", "q",
                                                   QRC, t)
                    rope_ps = {}
                    for m in range(QRC):
                        ps = qbps.tile([P, TT], F32, tag="qbps",
                                       name=f"qb{t}_{m}")
                        for k in range(QRC):
                            nc.tensor.matmul(ps[:], qbw[:, k, m * P:(m + 1) * P],
                                             qn[k][:], start=(k == 0),
                                             stop=(k == QRC - 1))
                        if m < NH:
                            o = evp.tile([P, TT], BF16, tag="ev")
                            nc.vector.tensor_copy(out=o[:], in_=ps[:])
                            nc.sync.dma_start(out=qnope_d[:, m, tsl], in_=o[:])
                        else:
                            rope_ps[m - NH] = ps
                    for ci in range(2):
                        lo_o, hi_o = rope_evict(rope_ps[ci], rope_ps[2 + ci],
                                                tsl, evp, cos_sb, sin_sb, "ev")
                        rope_store(lo_o, hi_o, ci, qrope_d, tsl)
            qbwp.release()
            qnp.release()
            ropep.release()
            # attention j=0 inputs: enqueue at B's tail so they load while
            # B's last tiles compute (knope/v ready since C, qnope at B end)
            hp = tc.alloc_tile_pool(name="ath", bufs=2, side="right")
            if phmax >= 5:
                pair_pre["k0"] = load_pair_k(0)
                pair_pre["q0"] = load_pair_q(0)

            # ------- Phase D: attention (head pairs, rope row-packed) -------
            with tc.tile_pool(name="atex", bufs=10) as exp_p, \
                 tc.tile_pool(name="atsum", bufs=6) as sump, \
                 tc.tile_pool(name="atev", bufs=6) as evp, \
                 tc.tile_pool(name="atsc", bufs=4, space="PSUM") as scp, \
                 tc.tile_pool(name="atpv", bufs=1, space="PSUM") as pvp:
                for j in range(NH // 2 if phmax >= 5 else 0):
                    h0, h1 = 2 * j, 2 * j + 1
                    if j == 0:
                        kn0, kn1, krj, v0, v1 = pair_pre["k0"]
                        qn0, qn1, qrj = pair_pre["q0"]
                    else:
                        kn0, kn1, krj, v0, v1 = load_pair_k(j)
                        qn0, qn1, qrj = load_pair_q(j)
                    if j == 0:
                        # o_proj weights for phase E load during attention
                        nc.sync.dma_start(out=oww[:], in_=ow_ap[:, :, :])
                    def mk_chain(qt):
                        """One query-tile attention chain; returns
                        (scores, pv, tail) closures over its own state."""
                        qsl = slice(qt * TT, (qt + 1) * TT)
                        sfx = qt % 2
                        pv0 = pvp.tile([P, TT], F32, tag=f"pv0{sfx}",
                                       name=f"pv0_{j}_{qt}")
                        pv1 = pvp.tile([P, TT], F32, tag=f"pv1{sfx}",
                                       name=f"pv1_{j}_{qt}")
                        # softmax denominator: bf16 pairwise tree over the exp
                        # tiles (binary counter of partials per head) - no
                        # serial chain, 2x DVE 16-bit mode
                        partials = ([], [])

                        def tree_push(hi, t):
                            lvl = 0
                            ps = partials[hi]
                            while ps and ps[-1][0] == lvl:
                                _, prev = ps.pop()
                                o = sump.tile([P, TT], BF16,
                                              tag=f"tr{hi}{sfx}",
                                              name=f"tr{hi}_{j}_{qt}_{lvl}")
                                nc.vector.tensor_add(out=o[:], in0=prev[:],
                                                     in1=t[:])
                                t = o
                                lvl += 1
                            ps.append((lvl, t))

                        def scores(kt):
                            ksl = slice(kt * P, (kt + 1) * P)
                            sc0 = scp.tile([P, TT], F32, tag="sc",
                                           name=f"sc0_{j}_{qt}_{kt}")
                            sc1 = scp.tile([P, TT], F32, tag="sc",
                                           name=f"sc1_{j}_{qt}_{kt}")
                            nc.tensor.matmul(sc0[:], kn0[:, ksl], qn0[:, qsl],
                                             start=True, stop=False)
                            nc.tensor.matmul(sc1[:], kn1[:, ksl], qn1[:, qsl],
                                             start=True, stop=False)
                            # K=64 rope matmuls: disjoint row groups run
                            # concurrently in the PE array (tile_position)
                            nc.tensor.matmul(sc0[:], krj[0:64, ksl],
                                             qrj[0:64, qsl],
                                             start=False, stop=True,
                                             tile_position=(0, 0))
                            nc.tensor.matmul(sc1[:], krj[64:128, ksl],
                                             qrj[64:128, qsl],
                                             start=False, stop=True,
                                             tile_position=(64, 0))
                            ex0 = exp_p.tile([P, TT], BF16, tag="ex",
                                             name=f"ex0_{j}_{qt}_{kt}")
                            ex1 = exp_p.tile([P, TT], BF16, tag="ex",
                                             name=f"ex1_{j}_{qt}_{kt}")
                            nc.scalar.activation(ex0[:], sc0[:], EXP_FN,
                                                 scale=SCALE)
                            nc.scalar.activation(ex1[:], sc1[:], EXP_FN,
                                                 scale=SCALE)
                            tree_push(0, ex0)
                            tree_push(1, ex1)
                            return ex0, ex1

                        def pv(kt, ex0, ex1):
                            st, sp = kt == 0, kt == nkt - 1
                            nc.tensor.matmul(pv0[:], v0[:, kt, :], ex0[:],
                                             start=st, stop=sp)
                            nc.tensor.matmul(pv1[:], v1[:, kt, :], ex1[:],
                                             start=st, stop=sp)

                        def tail():
                            for hi, (h, pvt) in enumerate(((h0, pv0),
                                                           (h1, pv1))):
                                # drain the tree (nkt power of two -> 1 entry)
                                ps = partials[hi]
                                while len(ps) > 1:
                                    _, a = ps.pop()
                                    _, b = ps.pop()
                                    o = sump.tile([P, TT], BF16,
                                                  tag=f"tr{hi}{sfx}",
                                                  name=f"trd{hi}_{j}_{qt}")
                                    nc.vector.tensor_add(out=o[:], in0=a[:],
                                                         in1=b[:])
                                    ps.append((99, o))
                                es = ps.pop()[1]
                                den = scp.tile([P, TT], F32, tag="sc",
                                               name=f"den_{j}_{qt}_{h}")
                                nc.tensor.matmul(den[:], ones_b[:], es[:],
                                                 start=True, stop=True)
                                recip = evp.tile([P, TT], F32, tag="evr",
                                                 name="recip")
                                # den in [~1, ~3e3]: approx_fast's 18 bits
                                # are far below bf16 noise, ~5x cheaper
                                nc.vector.reciprocal_approx_fast(recip[:],
                                                                 den[:])
                                ao = evp.tile([P, TT], BF16, tag="ev",
                                              name="ao")
                                nc.vector.tensor_mul(out=ao[:], in0=pvt[:],
                                                     in1=recip[:])
                                nc.sync.dma_start(out=attn_d[:, h, qsl],
                                                  in_=ao[:])
                        return scores, pv, tail

                    # two query-tile chains interleaved per head pair: the PE
                    # always has the other chain's scores between a chain's
                    # exp and its pv, so it never waits on the ACT engine
                    qts = list(range(ntt))
                    while qts:
                        if len(qts) >= 2:
                            sa, pa, ta = mk_chain(qts.pop(0))
                            sb, pb, tb_ = mk_chain(qts.pop(0))
                            exa = [sa(0)]
                            exb = [sb(0)]
                            for kt in range(1, nkt):
                                exa.append(sa(kt))
                                pa(kt - 1, *exa.pop(0))
                                exb.append(sb(kt))
                                pb(kt - 1, *exb.pop(0))
                            pa(nkt - 1, *exa.pop(0))
                            pb(nkt - 1, *exb.pop(0))
                            ta()
                            tb_()
                        else:
                            sa, pa, ta = mk_chain(qts.pop(0))
                            exa = [sa(0), sa(1)]
                            for kt in range(2, nkt):
                                exa.append(sa(kt))
                                pa(kt - 2, *exa.pop(0))
                            pa(nkt - 2, *exa.pop(0))
                            pa(nkt - 1, *exa.pop(0))
                            ta()

            hp.release()

            # ------- Phase E: o_proj (partial) ------------------------------
            with tc.tile_pool(name="oin", bufs=4) as inp, \
                 tc.tile_pool(name="oev", bufs=4) as evp, \
                 tc.tile_pool(name="ops", bufs=6, space="PSUM") as ops:
                for t8 in range(ntc if phmax >= 6 else 0):
                    at = inp.tile([P, NH, P], BF16, tag="at", name=f"at{t8}")
                    nc.sync.dma_start(out=at[:],
                                      in_=attn_d[:, :, t8 * P:(t8 + 1) * P])
                    for n in range(HID // TT):
                        ps = ops.tile([P, TT], F32, tag="ops", name=f"o{t8}_{n}")
                        for k in range(NH * DV // P):
                            nc.tensor.matmul(ps[:], at[:, k, :],
                                             oww[:, k, n * TT:(n + 1) * TT],
                                             start=(k == 0),
                                             stop=(k == NH * DV // P - 1))
                        o = evp.tile([P, TT], BF16, tag="ev")
                        nc.vector.tensor_copy(out=o[:], in_=ps[:])
                        nc.sync.dma_start(
                            out=out_part[t8 * P:(t8 + 1) * P,
                                         n * TT:(n + 1) * TT],
                            in_=o[:])
            owp.release()

    nc.compile()
    return nc


# ---------------------------------------------------------------------------
# host-side packing
# ---------------------------------------------------------------------------

def _rope_tables():
    inv_freq = 1.0 / (10000.0 ** (np.arange(0, DR, 2, dtype=np.float32) / DR))
    t = np.arange(MAXP, dtype=np.float32)
    freqs = np.outer(t, inv_freq)
    emb = np.concatenate([freqs, freqs], axis=-1)
    return np.cos(emb).astype(np.float32), np.sin(emb).astype(np.float32)


def core_weights(g, q_a_w, q_a_ln_w, q_b_w, kv_a_w, kv_a_ln_w, kv_b_w,
                 k_rope_w, o_w):
    """Pack the weight set for head-group g (heads g*8 .. g*8+8)."""
    heads = range(g * NH, (g + 1) * NH)
    qb_eff = (q_b_w * q_a_ln_w[None, :]).astype(np.float32)
    kvb_eff = (kv_b_w * kv_a_ln_w[None, :]).astype(np.float32)

    nope_rows = np.concatenate(
        [np.arange(h * (DN + DR), h * (DN + DR) + DN) for h in heads])
    lo_rows = np.concatenate(
        [np.arange(h * (DN + DR) + DN, h * (DN + DR) + DN + 32) for h in heads])
    hi_rows = np.concatenate(
        [np.arange(h * (DN + DR) + DN + 32, h * (DN + DR) + DN + 64)
         for h in heads])
    qb_rows = np.concatenate([nope_rows, lo_rows, hi_rows])

    k_rows = np.concatenate(
        [np.arange(h * (DN + DV), h * (DN + DV) + DN) for h in heads])
    v_rows = np.concatenate(
        [np.arange(h * (DN + DV) + DN, (h + 1) * (DN + DV)) for h in heads])

    kr_lo = np.concatenate([np.arange(h * DR, h * DR + 32) for h in heads])
    kr_hi = np.concatenate([np.arange(h * DR + 32, (h + 1) * DR) for h in heads])
    kr_rows = np.concatenate([kr_lo, kr_hi])

    o_cols = np.concatenate([np.arange(h * DV, (h + 1) * DV) for h in heads])

    def c(a):
        return np.ascontiguousarray(a).astype(NP_BF16)

    return {
        "qa_wT": c(q_a_w.T),
        "kva_wT": c(kv_a_w.T),
        "kr_wT": c(k_rope_w[kr_rows].T),
        "qb_wT": c(qb_eff[qb_rows].T),
        "kvbk_wT": c(kvb_eff[k_rows].T),
        "kvbv_wT": c(kvb_eff[v_rows].T),
        "o_wT": c(o_w[:, o_cols].T),
    }


def core_inputs(c, hidden_states, position_ids, weight_sets, tb=S,
                seqpar=SEQPAR):
    b, g = c // 4, c % 4
    tta = tb // 4
    cos_tab, sin_tab = _rope_tables()
    pos = np.asarray(position_ids[b][:tb])
    cos_b = cos_tab[pos]  # [tb, DR]
    sin_b = sin_tab[pos]
    cos_rep = np.ascontiguousarray(np.tile(cos_b[:, :32].T, (4, 1))).astype(
        NP_BF16)
    sin_rep = np.ascontiguousarray(np.tile(sin_b[:, :32].T, (4, 1))).astype(
        NP_BF16)
    x = np.asarray(hidden_states[b][:tb], dtype=np.float32)
    xTb = np.ascontiguousarray(x.T).astype(NP_BF16)
    im = {"xT": xTb, "cos_rep": cos_rep, "sin_rep": sin_rep}
    if seqpar:
        im["xA"] = np.ascontiguousarray(xTb[:, g * tta:(g + 1) * tta])
    im.update(weight_sets[g])
    return im


_CACHE = {}


def _get_nc(tb=S):
    if tb not in _CACHE:
        _CACHE[tb] = build_nc(tb)
    return _CACHE[tb]


def kernel(hidden_states, position_ids, q_a_w, q_a_ln_w, q_b_w,
           kv_a_w, kv_a_ln_w, kv_b_w, k_rope_w, o_w):
    hidden_states = np.asarray(hidden_states, dtype=np.float32)
    weight_sets = [
        core_weights(g, np.asarray(q_a_w, np.float32),
                     np.asarray(q_a_ln_w, np.float32),
                     np.asarray(q_b_w, np.float32),
                     np.asarray(kv_a_w, np.float32),
                     np.asarray(kv_a_ln_w, np.float32),
                     np.asarray(kv_b_w, np.float32),
                     np.asarray(k_rope_w, np.float32),
                     np.asarray(o_w, np.float32))
        for g in range(4)
    ]
    in_maps = [core_inputs(c, hidden_states, position_ids, weight_sets)
               for c in range(NCORES)]
    nc = _get_nc()
    res = run_bass_kernel_spmd(nc, in_maps, core_ids=list(range(NCORES)))
    out = np.zeros((B, S, HID), dtype=np.float64)
    for c in range(NCORES):
        out[c // 4] += np.asarray(res.results[c]["out_part"], np.float32)
    return out.astype(np.float32)
